# revision 1
# baseline (speedup 1.0000x reference)
"""Distributed GCN kernel for 8 Trainium2 NeuronCores (Bass/Tile).

Strategy (destination-sharded graph parallelism):
- Nodes are row-partitioned 8 ways (6250/core, padded to 6272 = 49*128).
- Dense matmuls (x@w1, x@w2+b, x@w4+b) run row-sharded from a host-transposed
  x shard, in float32r (TF32-like) on the TensorEngine.         [launch A]
- spmm(adj, X) is computed destination-sharded: edges sorted by row are packed
  into 128-edge blocks; each block's source rows are fetched with the GPSIMD
  dma_gather (edges -> partitions) and contracted with a host-built bf16
  "selection matrix" (one-hot by window offset, scaled by edge_val) on the
  TensorEngine, accumulating windows of 64 destination rows in PSUM.
  Layer 1 gathers from the bf16 x@w1 table, then h = RReLU(spmm + skip),
  hT via PE transpose, hw3 = h @ w3.                            [launch B]
  Layer 2 gathers from the bf16 hw3 table (padded to 128 cols) and adds the
  skip path to produce the output.                              [launch C]
- The gather tables (x@w1, h@w3: rows needed by every core) are exchanged
  between launches on the host (ncfw collectives cost ~8 ms fixed in this
  environment, so host-mediated all-gather of the 3-26 MB tables is used).
- dma_gather limits honored: int16 indices (tables split in two 25000-row
  halves), <=1024 indices per call (8 blocks).

Numerics: gather tables and selection values in bf16, PSUM accumulation and
skip paths in fp32, dense matmuls in float32r. End-to-end max error vs the
fp32 reference is ~6e-4 of the output scale.
"""
import sys
sys.path.insert(0, '/opt/trn_rl_repo')
import numpy as np
import ml_dtypes

import concourse.bacc as bacc
import concourse.tile as tile
import concourse.mybir as mybir
from concourse.masks import make_identity

bf16 = ml_dtypes.bfloat16
f32 = mybir.dt.float32
f32r = mybir.dt.float32r
bf = mybir.dt.bfloat16
i16 = mybir.dt.int16

# ---- problem / sharding constants (hardcoded per contract) ----
N = 50000
NFEAT = 512
NHID = 256
NCLASS = 64
NC = 8
SH = N // NC            # 6250 rows per core
NT = 49                 # node tiles of 128 rows per core
SHP = NT * 128          # 6272 padded rows
W = 64                  # PSUM window rows
NWIN = (SH + W - 1) // W  # 98 windows per core
B_HALF = 5              # blocks per (window, table-half) — global max fits
NBH = NWIN * B_HALF     # 490 blocks per half-stream
RHO = 25000             # gather-table split point (int16 index limit)
GBLK = 8                # blocks per dma_gather call (1024-idx SWDGE limit)
IDXCOLS = NBH * 8
RRELU_SLOPE = (1.0 / 8.0 + 1.0 / 3.0) / 2.0


# ================= host-side preprocessing =================

def _wrap_idx16(idx_flat):
    """dma_gather index layout: per call, [16, n/16] wrapped; calls
    concatenated along columns; replicated to 128 partitions."""
    cols = []
    pos = 0
    for k in range((NBH + GBLK - 1) // GBLK):
        nb = min(GBLK, NBH - k * GBLK)
        n = nb * 128
        cols.append(idx_flat[pos:pos + n].reshape(n // 16, 16).T)
        pos += n
    out16 = np.concatenate(cols, axis=1)
    return np.tile(out16, (8, 1)).copy()


def _build_edge_structure(edge_row, edge_col, edge_val):
    row = np.asarray(edge_row).astype(np.int64)
    col = np.asarray(edge_col).astype(np.int64)
    val = np.asarray(edge_val).astype(np.float32)

    core = row // SH
    r_local = row - core * SH
    win = r_local // W
    off = r_local % W
    half = (col >= RHO).astype(np.int64)
    colh = col - half * RHO

    idxw_all, S_all = [], []
    for c in range(NC):
        m = core == c
        idxw_c, S_c = [], []
        for h in (0, 1):
            mh = m & (half == h)
            w_h, off_h, col_h, val_h = win[mh], off[mh], colh[mh], val[mh]
            order = np.argsort(w_h, kind="stable")
            w_s, off_s, col_s, val_s = (w_h[order], off_h[order],
                                        col_h[order], val_h[order])
            cnt = np.bincount(w_s, minlength=NWIN)
            assert cnt.max() <= B_HALF * 128, (c, h, cnt.max())
            starts = np.zeros(NWIN, dtype=np.int64)
            starts[1:] = np.cumsum(cnt)[:-1]
            j = np.arange(len(w_s)) - starts[w_s]
            blk = w_s * B_HALF + j // 128
            p = j % 128
            idx_flat = np.zeros(NBH * 128, dtype=np.int16)
            idx_flat[blk * 128 + p] = col_s.astype(np.int16)
            S = np.zeros((128, NBH, W), dtype=bf16)
            S[p, blk, off_s] = val_s.astype(bf16)
            idxw_c.append(_wrap_idx16(idx_flat))
            S_c.append(S)
        idxw_all.append(idxw_c)
        S_all.append(S_c)
    return idxw_all, S_all


# ================= bass kernel builders =================

def build_A():
    nc = bacc.Bacc(num_devices=NC)
    xT = nc.dram_tensor("xT", [512, SHP], f32r, kind="ExternalInput")
    Wcat = nc.dram_tensor("Wcat", [512, 576], f32r, kind="ExternalInput")
    BB1 = nc.dram_tensor("BB1", [128, 256], f32, kind="ExternalInput")
    BB3 = nc.dram_tensor("BB3", [128, 64], f32, kind="ExternalInput")
    xw1 = nc.dram_tensor("xw1", [SHP, 256], bf, kind="ExternalOutput")
    sk0 = nc.dram_tensor("sk0", [SHP, 256], f32, kind="ExternalOutput")
    sk1 = nc.dram_tensor("sk1", [SHP, 64], f32, kind="ExternalOutput")

    with tile.TileContext(nc) as tc:
        with tc.tile_pool(name="per", bufs=1) as per, \
             tc.tile_pool(name="sb", bufs=3) as sb, \
             tc.tile_pool(name="ps", bufs=2, space="PSUM") as ps, \
             tc.tile_pool(name="psb", bufs=2, space="PSUM") as psb:
            xT_sb = []
            for fc in range(4):
                t = per.tile([128, SHP], f32r, name=f"xT{fc}", tag=f"xT{fc}")
                nc.sync.dma_start(t[:], xT[fc * 128:(fc + 1) * 128, :])
                xT_sb.append(t)
            W_sb = []
            for fc in range(4):
                t = per.tile([128, 576], f32r, name=f"W{fc}", tag=f"W{fc}")
                nc.sync.dma_start(t[:], Wcat[fc * 128:(fc + 1) * 128, :])
                W_sb.append(t)
            bb1 = per.tile([128, 256], f32)
            nc.sync.dma_start(bb1[:], BB1[:])
            bb3 = per.tile([128, 64], f32)
            nc.sync.dma_start(bb3[:], BB3[:])

            for t in range(NT):
                pa = ps.tile([128, 512], f32, space="PSUM", name=f"pa{t}", tag="pa")
                pb = psb.tile([128, 64], f32, space="PSUM", name=f"pb{t}", tag="pb")
                for fc in range(4):
                    lhs = xT_sb[fc][:, t * 128:(t + 1) * 128]
                    nc.tensor.matmul(pa[:], lhsT=lhs, rhs=W_sb[fc][:, :512],
                                     start=(fc == 0), stop=(fc == 3))
                    nc.tensor.matmul(pb[:], lhsT=lhs, rhs=W_sb[fc][:, 512:576],
                                     start=(fc == 0), stop=(fc == 3))
                xw1_t = sb.tile([128, 256], bf, name=f"x1{t}", tag="xw1")
                nc.vector.tensor_copy(xw1_t[:], pa[:, 0:256])
                nc.sync.dma_start(xw1[t * 128:(t + 1) * 128, :], xw1_t[:])
                sk0_t = sb.tile([128, 256], f32, name=f"s0{t}", tag="sk0")
                nc.vector.tensor_tensor(out=sk0_t[:], in0=pa[:, 256:512],
                                        in1=bb1[:], op=mybir.AluOpType.add)
                nc.sync.dma_start(sk0[t * 128:(t + 1) * 128, :], sk0_t[:])
                sk1_t = sb.tile([128, 64], f32, name=f"s1{t}", tag="sk1")
                nc.vector.tensor_tensor(out=sk1_t[:], in0=pb[:],
                                        in1=bb3[:], op=mybir.AluOpType.add)
                nc.sync.dma_start(sk1[t * 128:(t + 1) * 128, :], sk1_t[:])
    return nc


def _spmm_phase(nc, pools, table, idx_sb, S_dram, elem, nfree, out_cb):
    """Gather + selection-matmul pipeline (see module docstring)."""
    sbG, sbS, psw = pools
    G_tiles = [{}, {}]
    S_tiles = [{}, {}]
    issued = [-1, -1]

    def ensure_call(h, k):
        if k <= issued[h]:
            return
        issued[h] = k
        nb = min(GBLK, NBH - k * GBLK)
        G = sbG.tile([128, GBLK, elem], bf, name=f"G{h}_{k}", tag=f"G{h}")
        nc.gpsimd.dma_gather(
            out_ap=G[:, :nb, :],
            in_ap=table[h],
            idxs_ap=idx_sb[h][:, k * GBLK * 8: k * GBLK * 8 + nb * 8],
            num_idxs=nb * 128,
            num_idxs_reg=nb * 128,
            elem_size=elem,
        )
        St = sbS.tile([128, GBLK, 64], bf, name=f"St{h}_{k}", tag=f"S{h}")
        nc.sync.dma_start(St[:, :nb, :], S_dram[h][:, k * GBLK: k * GBLK + nb, :])
        G_tiles[h][k] = G
        S_tiles[h][k] = St

    for t in range(NT):
        for h in (0, 1):
            for k in range((10 * t) // GBLK, (10 * t + 9) // GBLK + 1):
                ensure_call(h, k)
        pw = psw.tile([128, nfree], f32, space="PSUM", name=f"pw{t}", tag="pw")
        for dw in range(2):
            w = 2 * t + dw
            for h in (0, 1):
                for j in range(B_HALF):
                    b = w * B_HALF + j
                    k, slot = b // GBLK, b % GBLK
                    nc.tensor.matmul(
                        pw[dw * 64:(dw + 1) * 64, :],
                        lhsT=S_tiles[h][k][:, slot, :],
                        rhs=G_tiles[h][k][:, slot, :nfree],
                        start=(h == 0 and j == 0),
                        stop=(h == 1 and j == B_HALF - 1),
                        skip_group_check=True,
                    )
        out_cb(t, pw)


def build_B():
    nc = bacc.Bacc(num_devices=NC)
    XW1 = nc.dram_tensor("XW1", [N, 256], bf, kind="ExternalInput")
    sk0 = nc.dram_tensor("sk0", [SHP, 256], f32, kind="ExternalInput")
    idx0 = nc.dram_tensor("idx0", [128, IDXCOLS], i16, kind="ExternalInput")
    idx1 = nc.dram_tensor("idx1", [128, IDXCOLS], i16, kind="ExternalInput")
    S0 = nc.dram_tensor("S0", [128, NBH, 64], bf, kind="ExternalInput")
    S1 = nc.dram_tensor("S1", [128, NBH, 64], bf, kind="ExternalInput")
    w3b = nc.dram_tensor("w3b", [256, 64], bf, kind="ExternalInput")
    hw3p = nc.dram_tensor("hw3p", [SHP, 128], bf, kind="ExternalOutput")

    with tile.TileContext(nc) as tc:
        with tc.tile_pool(name="per", bufs=1) as per, \
             tc.tile_pool(name="sbG", bufs=3) as sbG, \
             tc.tile_pool(name="sbS", bufs=3) as sbS, \
             tc.tile_pool(name="sbv", bufs=3) as sbv, \
             tc.tile_pool(name="psw", bufs=3, space="PSUM") as psw, \
             tc.tile_pool(name="pst", bufs=2, space="PSUM") as pst, \
             tc.tile_pool(name="pso", bufs=2, space="PSUM") as pso:

            idx_sb = []
            for nm, tsr in (("i0", idx0), ("i1", idx1)):
                t = per.tile([128, IDXCOLS], i16, name=nm, tag=nm)
                nc.sync.dma_start(t[:], tsr[:])
                idx_sb.append(t)
            w3_sb = per.tile([128, 2, 64], bf, tag="w3")
            nc.sync.dma_start(w3_sb[:, 0, :], w3b[0:128, :])
            nc.sync.dma_start(w3_sb[:, 1, :], w3b[128:256, :])
            ident = per.tile([128, 128], bf, tag="ident")
            make_identity(nc, ident[:])

            def evac_B(t, pw):
                sk = sbv.tile([128, 256], f32, name=f"sk{t}", tag="sk")
                nc.sync.dma_start(sk[:], sk0[t * 128:(t + 1) * 128, :])
                v = sbv.tile([128, 256], f32, name=f"v{t}", tag="v")
                nc.vector.tensor_tensor(out=v[:], in0=pw[:], in1=sk[:],
                                        op=mybir.AluOpType.add)
                r = sbv.tile([128, 256], f32, name=f"r{t}", tag="r")
                nc.scalar.activation(r[:], v[:],
                                     mybir.ActivationFunctionType.Relu,
                                     scale=1.0 - RRELU_SLOPE)
                ht = sbv.tile([128, 256], bf, name=f"h{t}", tag="h")
                nc.vector.scalar_tensor_tensor(
                    out=ht[:], in0=v[:], scalar=RRELU_SLOPE, in1=r[:],
                    op0=mybir.AluOpType.mult, op1=mybir.AluOpType.add)
                hT = sbv.tile([128, 2, 128], bf, name=f"hT{t}", tag="hT")
                for fc in range(2):
                    pt = pst.tile([128, 128], bf, space="PSUM",
                                  name=f"pt{t}_{fc}", tag="pt")
                    nc.tensor.transpose(pt[:], ht[:, fc * 128:(fc + 1) * 128],
                                        ident[:])
                    nc.vector.tensor_copy(hT[:, fc, :], pt[:])
                po = pso.tile([128, 64], f32, space="PSUM",
                              name=f"po{t}", tag="po")
                for fc in range(2):
                    nc.tensor.matmul(po[:], lhsT=hT[:, fc, :],
                                     rhs=w3_sb[:, fc, :],
                                     start=(fc == 0), stop=(fc == 1))
                h3 = sbv.tile([128, 128], bf, name=f"h3{t}", tag="h3")
                nc.vector.memset(h3[:, 64:128], 0.0)
                nc.vector.tensor_copy(h3[:, 0:64], po[:])
                nc.sync.dma_start(hw3p[t * 128:(t + 1) * 128, :], h3[:])

            _spmm_phase(nc, (sbG, sbS, psw),
                        (XW1[0:RHO, :], XW1[RHO:N, :]),
                        idx_sb, (S0, S1), 256, 256, evac_B)
    return nc


def build_C():
    nc = bacc.Bacc(num_devices=NC)
    HW3P = nc.dram_tensor("HW3P", [N, 128], bf, kind="ExternalInput")
    sk1 = nc.dram_tensor("sk1", [SHP, 64], f32, kind="ExternalInput")
    idx0 = nc.dram_tensor("idx0", [128, IDXCOLS], i16, kind="ExternalInput")
    idx1 = nc.dram_tensor("idx1", [128, IDXCOLS], i16, kind="ExternalInput")
    S0 = nc.dram_tensor("S0", [128, NBH, 64], bf, kind="ExternalInput")
    S1 = nc.dram_tensor("S1", [128, NBH, 64], bf, kind="ExternalInput")
    outp = nc.dram_tensor("outp", [SHP, 64], f32, kind="ExternalOutput")

    with tile.TileContext(nc) as tc:
        with tc.tile_pool(name="per", bufs=1) as per, \
             tc.tile_pool(name="sbG", bufs=3) as sbG, \
             tc.tile_pool(name="sbS", bufs=3) as sbS, \
             tc.tile_pool(name="sbv", bufs=3) as sbv, \
             tc.tile_pool(name="psw", bufs=4, space="PSUM") as psw:
            idx_sb = []
            for nm, tsr in (("i0", idx0), ("i1", idx1)):
                t = per.tile([128, IDXCOLS], i16, name=nm, tag=nm)
                nc.sync.dma_start(t[:], tsr[:])
                idx_sb.append(t)

            def evac_C(t, pw):
                sk = sbv.tile([128, 64], f32, name=f"skc{t}", tag="skc")
                nc.sync.dma_start(sk[:], sk1[t * 128:(t + 1) * 128, :])
                ot = sbv.tile([128, 64], f32, name=f"ot{t}", tag="ot")
                nc.vector.tensor_tensor(out=ot[:], in0=pw[:], in1=sk[:],
                                        op=mybir.AluOpType.add)
                nc.sync.dma_start(outp[t * 128:(t + 1) * 128, :], ot[:])

            _spmm_phase(nc, (sbG, sbS, psw),
                        (HW3P[0:RHO, :], HW3P[RHO:N, :]),
                        idx_sb, (S0, S1), 128, 64, evac_C)
    return nc


# ================= SPMD runner (PJRT via axon) =================

class _SpmdRunner:
    def __init__(self, nc, n_cores=NC):
        import jax
        from jax.sharding import Mesh, PartitionSpec
        from jax.experimental.shard_map import shard_map
        from concourse import bass2jax
        from concourse.bass2jax import _bass_exec_p, install_neuronx_cc_hook
        install_neuronx_cc_hook()
        if not nc.is_finalized():
            nc.finalize()
        self.n_cores = n_cores
        partition_name = (nc.partition_id_tensor.name
                          if nc.partition_id_tensor else None)
        in_names, out_names, out_avals, zero_outs = [], [], [], []
        for alloc in nc.m.functions[0].allocations:
            if not isinstance(alloc, mybir.MemoryLocationSet):
                continue
            name = alloc.memorylocations[0].name
            if alloc.kind == "ExternalInput":
                if name != partition_name:
                    in_names.append(name)
            elif alloc.kind == "ExternalOutput":
                out_names.append(name)
                shape = tuple(alloc.tensor_shape)
                dtype = mybir.dt.np(alloc.dtype)
                out_avals.append(jax.core.ShapedArray(shape, dtype))
                zero_outs.append(np.zeros(shape, dtype))
        self.in_names, self.out_names = in_names, out_names
        self.out_avals, self.zero_outs = out_avals, zero_outs
        n_params = len(in_names)
        self.n_params = n_params
        all_in_names = list(in_names) + list(out_names)
        if partition_name is not None:
            all_in_names.append(partition_name)

        def _body(*args):
            operands = list(args)
            if partition_name is not None:
                operands.append(bass2jax.partition_id_tensor())
            outs = _bass_exec_p.bind(
                *operands,
                out_avals=tuple(out_avals),
                in_names=tuple(all_in_names),
                out_names=tuple(out_names),
                lowering_input_output_aliases=(),
                sim_require_finite=True,
                sim_require_nnan=True,
                nc=nc,
            )
            return tuple(outs)

        devices = jax.devices()[:n_cores]
        mesh = Mesh(np.asarray(devices), ("core",))
        in_specs = (PartitionSpec("core"),) * (n_params + len(out_names))
        out_specs = (PartitionSpec("core"),) * len(out_names)
        self.fn = jax.jit(
            shard_map(_body, mesh=mesh, in_specs=in_specs,
                      out_specs=out_specs, check_rep=False),
            keep_unused=True,
        )

    def run(self, in_maps):
        import jax
        per_core = [[np.asarray(m[name]) for name in self.in_names]
                    for m in in_maps]
        concat_in = [np.concatenate([per_core[c][i] for c in range(self.n_cores)],
                                    axis=0) for i in range(self.n_params)]
        concat_zeros = [np.zeros((self.n_cores * z.shape[0], *z.shape[1:]), z.dtype)
                        for z in self.zero_outs]
        out_arrs = self.fn(*(concat_in + concat_zeros))
        jax.block_until_ready(out_arrs)
        return [
            {name: np.asarray(out_arrs[i]).reshape(self.n_cores,
                                                   *self.out_avals[i].shape)[c]
             for i, name in enumerate(self.out_names)}
            for c in range(self.n_cores)
        ]


_CACHE = {}


def _get_runners():
    if "runners" not in _CACHE:
        _CACHE["runners"] = (_SpmdRunner(build_A()), _SpmdRunner(build_B()),
                             _SpmdRunner(build_C()))
    return _CACHE["runners"]


# ================= public entry point =================

def kernel(x, edge_row, edge_col, edge_val, w1, b1, w2, b2, w3, b3, w4, b4):
    x = np.asarray(x, dtype=np.float32)
    w1 = np.asarray(w1, dtype=np.float32)
    w2 = np.asarray(w2, dtype=np.float32)
    w3 = np.asarray(w3, dtype=np.float32)
    w4 = np.asarray(w4, dtype=np.float32)
    b1 = np.asarray(b1, dtype=np.float32)
    b2 = np.asarray(b2, dtype=np.float32)
    b3 = np.asarray(b3, dtype=np.float32)
    b4 = np.asarray(b4, dtype=np.float32)

    idxw, S = _build_edge_structure(edge_row, edge_col, edge_val)
    Wcat = np.concatenate([w1, w2, w4], axis=1).copy()
    BB1 = np.tile((b1 + b2)[None, :], (128, 1)).copy()
    BB3 = np.tile((b3 + b4)[None, :], (128, 1)).copy()
    w3b = w3.astype(bf16)
    xT_pad = []
    for c in range(NC):
        xp = np.zeros((512, SHP), np.float32)
        xp[:, :SH] = x[c * SH:(c + 1) * SH, :].T
        xT_pad.append(xp)

    rA, rB, rC = _get_runners()

    resA = rA.run([{"xT": xT_pad[c], "Wcat": Wcat, "BB1": BB1, "BB3": BB3}
                   for c in range(NC)])

    def as_bf16(a):
        return a.view(bf16) if a.dtype.itemsize == 2 and a.dtype != np.float16 else a

    XW1 = np.ascontiguousarray(np.concatenate(
        [as_bf16(resA[c]["xw1"])[:SH] for c in range(NC)], axis=0))
    sk0_list = [resA[c]["sk0"] for c in range(NC)]
    sk1_list = [resA[c]["sk1"] for c in range(NC)]

    resB = rB.run([{"XW1": XW1, "sk0": sk0_list[c],
                    "idx0": idxw[c][0], "idx1": idxw[c][1],
                    "S0": S[c][0], "S1": S[c][1], "w3b": w3b}
                   for c in range(NC)])
    HW3P = np.ascontiguousarray(np.concatenate(
        [as_bf16(resB[c]["hw3p"])[:SH] for c in range(NC)], axis=0))

    resC = rC.run([{"HW3P": HW3P, "sk1": sk1_list[c],
                    "idx0": idxw[c][0], "idx1": idxw[c][1],
                    "S0": S[c][0], "S1": S[c][1]}
                   for c in range(NC)])
    out = np.concatenate([resC[c]["outp"][:SH] for c in range(NC)], axis=0)

    # reference returns (out, w1, w2, w3, w4); pass weights through unchanged
    return (out,
            np.asarray(w1, dtype=np.float32),
            np.asarray(w2, dtype=np.float32),
            np.asarray(w3, dtype=np.float32),
            np.asarray(w4, dtype=np.float32))
